# revision 8
# baseline (speedup 1.0000x reference)
"""Trainium2 Bass kernel for causal MLA self-attention.

Problem: B=2, T=2048, C=2048, H=16 heads, Dh=128, latent Dl=64.
  q = rope(x @ wq); k_lat = rope(x @ wk_lat); v_lat = x @ wv_lat
  k_h = k_lat @ k_expand[h]; v_h = v_lat @ v_expand[h]
  y = causal_softmax(q k^T / sqrt(Dh)) v;  out = y @ proj_w

Sharding: 8 cores = 2 batches x 4 head-groups (4 heads each).  Each core
computes a full (T, C) partial of the output projection restricted to its
heads; the host sums the 4 partials per batch.

Device algorithm (per core), MLA absorption + W2 fusion:
  qt_h = rope(q_h) @ k_expand[h]^T             (T, 64) bf16
  s^T  = kk_rope @ qt_h^T                      (Tk, Tq) tiles; exp on ScalarE
  yu   = ex^T [stationary] @ [v_lat | 1]       (Tq128, 65) natural layout
  yun  = yu[:, :64] * recip(yu[:, 64])         per-partition scalar mul
  yt   = yun^T (PE transpose, 2 heads stacked) (128, Tq)
  out += yt2^T @ W2  where W2 = v_expand @ proj_w (fused, built on device)
All big matmul *moving* operands are bf16 (cost-model 1 cyc/row at any
width); accumulation stays fp32 in PSUM.
"""

import os
import sys

import numpy as np
import ml_dtypes

if not any(os.path.isdir(os.path.join(p, "concourse")) for p in sys.path if p):
    sys.path.insert(0, "/opt/trn_rl_repo")

import concourse.bass as bass  # noqa: E402
import concourse.mybir as mybir  # noqa: E402
import concourse.tile as tile  # noqa: E402
from concourse import bacc  # noqa: E402
from concourse.bass_utils import run_bass_kernel_spmd  # noqa: E402

B, T, C, H, Dh, Dl = 2, 2048, 2048, 16, 128, 64
HPC = 4  # heads per core
NCORES = 8
F32 = mybir.dt.float32
F32R = mybir.dt.float32r
BF16 = mybir.dt.bfloat16
SCALE = 1.0 / float(np.sqrt(Dh))

TJ = 512          # Tq chunk
NJ = T // TJ      # 4
NK = C // 128     # 16 contraction chunks over C
NTK = T // 128    # 16 Tk chunks
EXP = mybir.ActivationFunctionType.Exp


def build_nc():
    nc = bacc.Bacc(None, target_bir_lowering=False, debug=False)

    xT = nc.dram_tensor("xT", [C, T], BF16, kind="ExternalInput")
    wq = nc.dram_tensor("wq", [C, HPC * Dh], BF16, kind="ExternalInput")
    wkv = nc.dram_tensor("wkv", [128, NK, 2 * Dl], BF16, kind="ExternalInput")
    eT = nc.dram_tensor("eT", [Dh, HPC * Dl], BF16, kind="ExternalInput")
    eT2 = nc.dram_tensor("eT2", [Dh, HPC * Dl], BF16, kind="ExternalInput")
    vxT = nc.dram_tensor("vxT", [Dh, HPC * Dl], BF16, kind="ExternalInput")
    pw = nc.dram_tensor("pw", [HPC * Dh, C], BF16, kind="ExternalInput")
    cosq = nc.dram_tensor("cosq", [Dh, T], F32, kind="ExternalInput")
    sinq = nc.dram_tensor("sinq", [Dh, T], F32, kind="ExternalInput")
    cosk = nc.dram_tensor("cosk", [Dl, T], F32, kind="ExternalInput")
    sink = nc.dram_tensor("sink", [Dl, T], F32, kind="ExternalInput")
    sperm = nc.dram_tensor("sperm", [Dl, Dl], F32R, kind="ExternalInput")
    ident = nc.dram_tensor("ident", [128, 128], BF16, kind="ExternalInput")
    maskt = nc.dram_tensor("maskt", [128, 4, TJ], BF16, kind="ExternalInput")
    out = nc.dram_tensor("out", [T, C], BF16, kind="ExternalOutput")

    with tile.TileContext(nc) as tc, \
         nc.allow_low_precision(reason="bf16 pipeline, fp32 accumulation"):
        cp = tc.alloc_tile_pool(name="consts", bufs=1)
        wp = tc.alloc_tile_pool(name="work", bufs=1)
        ps = tc.alloc_tile_pool(name="ps", bufs=1, space="PSUM")

        # ------- static tiles -------
        wq_sb = cp.tile([128, NK, HPC * Dh], BF16, name="wq_sb")
        wkv_sb = cp.tile([128, NK, 2 * Dl], BF16, name="wkv_sb")
        eT_sb = cp.tile([Dh, HPC * Dl], BF16, name="eT_sb")
        eT2_sb = cp.tile([Dh, HPC * Dl], BF16, name="eT2_sb")
        vxT_sb = cp.tile([Dh, HPC * Dl], BF16, name="vxT_sb")
        pw_sb = cp.tile([128, HPC, C], BF16, name="pw_sb")
        cosq_sb = cp.tile([Dh, T], F32, name="cosq_sb")
        sinq_sb = cp.tile([Dh, T], F32, name="sinq_sb")
        cosk_sb = cp.tile([Dl, T], F32, name="cosk_sb")
        sink_sb = cp.tile([Dl, T], F32, name="sink_sb")
        sperm_sb = cp.tile([Dl, Dl], F32R, name="sperm_sb")
        ident_sb = cp.tile([128, 128], BF16, name="ident_sb")
        maskt_sb = cp.tile([128, 4, TJ], BF16, name="maskt_sb")
        kk_sb = cp.tile([Dl, T], BF16, name="kk_sb")
        qtil_sb = cp.tile([Dl, HPC, T], BF16, name="qtil_sb")
        vaug_sb = cp.tile([128, NTK, Dl + 1], BF16, name="vaug_sb")
        w2_sb = cp.tile([128, 2, C], BF16, name="w2_sb")

        # ------- input DMAs (SP queue, in priority order) -------
        nc.sync.dma_start(wkv_sb, wkv[:])
        nc.sync.dma_start(cosk_sb, cosk[:])
        nc.sync.dma_start(sink_sb, sink[:])
        nc.sync.dma_start(sperm_sb, sperm[:])
        xr = xT[:].rearrange("(ko p) t -> p ko t", p=128)
        xts = []
        for j in range(NJ):
            xts.append(wp.tile([128, NK, TJ], BF16, name=f"xt{j}", tag="xt",
                               bufs=2))
        nc.sync.dma_start(xts[0], xr[:, :, 0:TJ])
        nc.sync.dma_start(wq_sb, wq[:].rearrange("(ko p) m -> p ko m", p=128))
        nc.sync.dma_start(cosq_sb, cosq[:])
        nc.sync.dma_start(sinq_sb, sinq[:])
        nc.sync.dma_start(eT_sb, eT[:])
        nc.sync.dma_start(eT2_sb, eT2[:])
        nc.sync.dma_start(maskt_sb, maskt[:])
        nc.sync.dma_start(ident_sb, ident[:])
        nc.sync.dma_start(vxT_sb, vxT[:])
        nc.sync.dma_start(pw_sb, pw[:].rearrange("(ko p) n -> p ko n", p=128))
        nc.vector.memset(vaug_sb[:, :, Dl:Dl + 1], 1.0)

        # ------- W2 = v_expand @ proj_w (fused output weight) -------
        for h in range(HPC):
            hs = slice((h % 2) * Dl, (h % 2) * Dl + Dl)
            for cc in range(4):
                cs = slice(cc * TJ, (cc + 1) * TJ)
                w2p = ps.tile([Dl, TJ], F32, name=f"w2p{h}_{cc}", tag="p64",
                              bufs=1)
                nc.tensor.matmul(w2p, vxT_sb[:, h * Dl:(h + 1) * Dl],
                                 pw_sb[:, h, cs], start=True, stop=True)
                nc.vector.tensor_copy(w2_sb[hs, h // 2, cs], w2p)

        # ------- main pipeline over Tq chunks -------
        for j in range(NJ):
            js = slice(j * TJ, (j + 1) * TJ)
            if j + 1 < NJ:
                nc.sync.dma_start(xts[j + 1],
                                  xr[:, :, (j + 1) * TJ:(j + 2) * TJ])

            # --- latent K/V projection + k-RoPE ---
            kvps = ps.tile([128, TJ], F32, name=f"kvps{j}", tag="p128", bufs=2)
            for k in range(NK):
                nc.tensor.matmul(kvps, wkv_sb[:, k, :], xts[j][:, k, :],
                                 start=(k == 0), stop=(k == NK - 1))
            klat = wp.tile([Dl, TJ], F32R, name=f"klat{j}", tag="klat", bufs=2)
            nc.vector.tensor_copy(klat, kvps[0:Dl, :])
            vt = wp.tile([Dl, TJ], BF16, name=f"vt{j}", tag="vt", bufs=2)
            nc.vector.tensor_copy(vt, kvps[Dl:128, :])
            ksps = ps.tile([Dl, TJ], F32, name=f"ksps{j}", tag="p64", bufs=1)
            nc.tensor.matmul(ksps, sperm_sb, klat, start=True, stop=True)
            tk1 = wp.tile([Dl, TJ], F32, name=f"tk1_{j}", tag="tk1", bufs=2)
            nc.gpsimd.tensor_mul(tk1, klat, cosk_sb[:, js])
            tk2 = wp.tile([Dl, TJ], F32, name=f"tk2_{j}", tag="tk2", bufs=2)
            nc.vector.tensor_mul(tk2, ksps, sink_sb[:, js])
            nc.gpsimd.tensor_add(kk_sb[:, js], tk1, tk2)
            for t in range(4):
                vtt = wp.tile([128, Dl], BF16, name=f"vtt{j}_{t}", tag="vtt",
                              bufs=3)
                nc.sync.dma_start_transpose(vtt, vt[:, t * 128:(t + 1) * 128])
                nc.gpsimd.tensor_copy(vaug_sb[:, 4 * j + t, 0:Dl], vtt)

            # --- q projection + fused RoPE/absorb ---
            for h in range(HPC):
                qp = ps.tile([128, TJ], F32, name=f"qp{j}_{h}", tag="p128",
                             bufs=2)
                for k in range(NK):
                    nc.tensor.matmul(qp, wq_sb[:, k, h * 128:(h + 1) * 128],
                                     xts[j][:, k, :],
                                     start=(k == 0), stop=(k == NK - 1))
                u1 = wp.tile([128, TJ], BF16, name=f"u1_{j}_{h}", tag="u1",
                             bufs=2)
                nc.vector.tensor_mul(u1, qp, cosq_sb[:, js])
                u2 = wp.tile([128, TJ], BF16, name=f"u2_{j}_{h}", tag="u2",
                             bufs=2)
                nc.vector.tensor_mul(u2, qp, sinq_sb[:, js])
                qtp = ps.tile([Dl, TJ], F32, name=f"qtp{j}_{h}", tag="p64",
                              bufs=1)
                msl = slice(h * Dl, (h + 1) * Dl)
                nc.tensor.matmul(qtp, eT_sb[:, msl], u1,
                                 start=True, stop=False)
                nc.tensor.matmul(qtp, eT2_sb[:, msl], u2,
                                 start=False, stop=True)
                nc.vector.tensor_copy(qtil_sb[:, h, js], qtp)

            # --- attention (scores + AV in natural layout) ---
            nm = 4 * (j + 1)
            for h in range(HPC):
                p, hf = h // 2, h % 2
                av = ps.tile([128, 4, Dl + 1], F32, name=f"av{j}_{h}",
                             tag="av", bufs=1)
                if hf == 0:
                    ytp = ps.tile([128, TJ], BF16, name=f"ytp{j}_{p}",
                                  tag="ytp", bufs=1)
                exs = []
                for m in range(nm):
                    ms = slice(m * 128, (m + 1) * 128)
                    d = m - 4 * j
                    lo = 128 * d if d > 0 else 0
                    sps = ps.tile([128, TJ], F32, name=f"sps{j}_{h}_{m}",
                                  tag="sps", bufs=1)
                    nc.tensor.matmul(sps[:, lo:TJ], kk_sb[:, ms],
                                     qtil_sb[:, h, j * TJ + lo:(j + 1) * TJ],
                                     start=True, stop=True)
                    ex = wp.tile([128, TJ], BF16, name=f"ex{j}_{h}_{m}",
                                 tag="ex", bufs=24)
                    nc.scalar.activation(ex[:, lo:TJ], sps[:, lo:TJ], EXP,
                                         scale=SCALE)
                    if d >= 0:
                        band = slice(lo, lo + 128)
                        eng = nc.vector if m % 2 else nc.gpsimd
                        eng.tensor_mul(ex[:, band], ex[:, band],
                                       maskt_sb[:, d, band])
                    exs.append(ex)
                # one open accumulation chain per PSUM bank at a time:
                # issue each tq-chunk's chain contiguously (chain-major)
                for t in range(4):
                    for m in range(4 * j + t + 1):
                        nc.tensor.matmul(
                            av[:, t, :], exs[m][:, t * 128:(t + 1) * 128],
                            vaug_sb[:, m, :],
                            start=(m == 0), stop=(m == 4 * j + t))
                # normalize + transpose into ytp (pair-stacked)
                rec = wp.tile([128, 4], F32, name=f"rec{j}_{h}", tag="rec",
                              bufs=2)
                nc.vector.reciprocal(rec, av[:, :, Dl])
                yun = wp.tile([128, 4, Dl], BF16, name=f"yun{j}_{h}",
                              tag="yun", bufs=2)
                for t in range(4):
                    nc.vector.tensor_scalar_mul(yun[:, t, :], av[:, t, 0:Dl],
                                                rec[:, t:t + 1])
                    nc.tensor.transpose(
                        ytp[hf * Dl:(hf + 1) * Dl, t * 128:(t + 1) * 128],
                        yun[:, t, :], ident_sb)
                if hf == 1:
                    yt2 = wp.tile([128, TJ], BF16, name=f"yt2_{j}_{p}",
                                  tag="yt2", bufs=2)
                    nc.vector.tensor_copy(yt2, ytp)
                    if p == 0:
                        yt2a = yt2
                    else:
                        yt2b = yt2

            # --- fused output projection for this j's 4 row chunks ---
            for t in range(4):
                ts = slice(t * 128, (t + 1) * 128)
                rs = slice((4 * j + t) * 128, (4 * j + t + 1) * 128)
                ot = wp.tile([128, C], BF16, name=f"ot{j}_{t}", tag="ot",
                             bufs=2)
                for cc in range(4):
                    cs = slice(cc * TJ, (cc + 1) * TJ)
                    pps = ps.tile([128, TJ], F32, name=f"pps{j}_{t}_{cc}",
                                  tag="pps", bufs=2)
                    nc.tensor.matmul(pps, yt2a[:, ts], w2_sb[:, 0, cs],
                                     start=True, stop=False)
                    nc.tensor.matmul(pps, yt2b[:, ts], w2_sb[:, 1, cs],
                                     start=False, stop=True)
                    if (t + cc) % 2:
                        nc.scalar.copy(ot[:, cs], pps)
                    else:
                        nc.vector.tensor_copy(ot[:, cs], pps)
                nc.scalar.dma_start(out[rs, :], ot)

        ps.release()
        wp.release()
        cp.release()

    nc.compile()
    return nc


def _rope_tables(t, d):
    inv = 1.0 / (10000.0 ** (np.arange(0, d, 2, dtype=np.float64) / d))
    ang = np.arange(t, dtype=np.float64)[:, None] * inv[None, :]  # (t, d/2)
    cos = np.cos(ang).T  # (d/2, t)
    sin = np.sin(ang).T
    cosf = np.empty((d, t), np.float32)
    sinf = np.empty((d, t), np.float32)
    cosf[0::2] = cos
    cosf[1::2] = cos
    sinf[0::2] = -sin
    sinf[1::2] = sin
    return cosf, sinf


def _host_inputs(x, wq, wk_lat, wv_lat, k_expand, v_expand, proj_w):
    bf = ml_dtypes.bfloat16
    cosq, sinq = _rope_tables(T, Dh)
    sinq = np.ascontiguousarray(sinq[np.arange(Dh) ^ 1, :])  # row-pair swap
    cosk, sink = _rope_tables(T, Dl)
    sperm = np.zeros((Dl, Dl), np.float32)
    idx = np.arange(Dl)
    sperm[idx, idx ^ 1] = 1.0
    idxh = np.arange(Dh)
    ident = np.eye(128, dtype=bf)
    tkr = np.arange(128)[:, None]
    tqr = np.arange(TJ)[None, :]
    maskt = np.stack(
        [(tkr + 128 * d <= tqr).astype(bf) for d in range(4)], axis=1)
    wkv = np.concatenate([wk_lat, wv_lat], axis=1).astype(bf)  # [C, 128]
    wkvp = np.ascontiguousarray(wkv.reshape(NK, 128, 2 * Dl).transpose(1, 0, 2))

    xTs = [np.ascontiguousarray(x[b].T.astype(bf)) for b in range(B)]
    in_maps = []
    for core in range(NCORES):
        b, g = divmod(core, 4)
        heads = range(4 * g, 4 * g + 4)
        eTc = np.ascontiguousarray(
            np.concatenate([k_expand[h].T for h in heads], axis=1).astype(bf))
        eT2c = np.ascontiguousarray(eTc[idxh ^ 1, :])
        vxTc = np.ascontiguousarray(
            np.concatenate([v_expand[h].T for h in heads], axis=1).astype(bf))
        in_maps.append({
            "xT": xTs[b],
            "wq": np.ascontiguousarray(
                wq[:, g * 512:(g + 1) * 512].astype(bf)),
            "wkv": wkvp,
            "eT": eTc, "eT2": eT2c, "vxT": vxTc,
            "pw": np.ascontiguousarray(
                proj_w[g * 512:(g + 1) * 512, :].astype(bf)),
            "cosq": cosq, "sinq": sinq, "cosk": cosk, "sink": sink,
            "sperm": sperm, "ident": ident, "maskt": maskt,
        })
    return in_maps


_NC_CACHE = {}


def run(inputs, trace=False, **kw):
    """Run on all 8 cores; returns (output, BassKernelResults)."""
    if "nc" not in _NC_CACHE:
        _NC_CACHE["nc"] = build_nc()
    nc = _NC_CACHE["nc"]
    in_maps = _host_inputs(**inputs)
    res = run_bass_kernel_spmd(
        nc, in_maps, core_ids=list(range(NCORES)), trace=trace, **kw)
    out = np.zeros((B, T, C), np.float32)
    for core in range(NCORES):
        out[core // 4] += res.results[core]["out"].astype(np.float32)
    return out, res


def kernel(**inputs):
    out, _ = run(inputs)
    return out


# revision 9
# speedup vs baseline: 1.1787x; 1.1787x over previous
"""Trainium2 Bass kernel for causal MLA self-attention.

Problem: B=2, T=2048, C=2048, H=16 heads, Dh=128, latent Dl=64.
  q = rope(x @ wq); k_lat = rope(x @ wk_lat); v_lat = x @ wv_lat
  k_h = k_lat @ k_expand[h]; v_h = v_lat @ v_expand[h]
  y = causal_softmax(q k^T / sqrt(Dh)) v;  out = y @ proj_w

Sharding: 8 cores = 2 batches x 4 head-groups (4 heads each).  Each core
computes a full (T, C) partial of the output projection restricted to its
heads; the host sums the 4 partials per batch.

Device algorithm (per core), MLA absorption + W2 fusion:
  qt_h = rope(q_h) @ k_expand[h]^T             (T, 64) bf16
  s^T  = kk_rope @ qt_h^T                      (Tk, Tq) tiles; exp on ScalarE
  yu   = ex^T [stationary] @ [v_lat | 1]       (Tq128, 65) natural layout
  yun  = yu[:, :64] * recip(yu[:, 64])         per-partition scalar mul
  yt   = yun^T (PE transpose, 2 heads stacked) (128, Tq)
  out += yt2^T @ W2  where W2 = v_expand @ proj_w (fused, built on device)
All big matmul *moving* operands are bf16 (cost-model 1 cyc/row at any
width); accumulation stays fp32 in PSUM.
"""

import os
import sys

import numpy as np
import ml_dtypes

if not any(os.path.isdir(os.path.join(p, "concourse")) for p in sys.path if p):
    sys.path.insert(0, "/opt/trn_rl_repo")

import concourse.bass as bass  # noqa: E402
import concourse.mybir as mybir  # noqa: E402
import concourse.tile as tile  # noqa: E402
from concourse import bacc  # noqa: E402
from concourse.bass_utils import run_bass_kernel_spmd  # noqa: E402

B, T, C, H, Dh, Dl = 2, 2048, 2048, 16, 128, 64
HPC = 4  # heads per core
NCORES = 8
F32 = mybir.dt.float32
F32R = mybir.dt.float32r
BF16 = mybir.dt.bfloat16
SCALE = 1.0 / float(np.sqrt(Dh))

TJ = 512          # Tq chunk
NJ = T // TJ      # 4
NK = C // 128     # 16 contraction chunks over C
NTK = T // 128    # 16 Tk chunks
EXP = mybir.ActivationFunctionType.Exp


def build_nc():
    nc = bacc.Bacc(None, target_bir_lowering=False, debug=False)

    xT = nc.dram_tensor("xT", [C, T], BF16, kind="ExternalInput")
    wq = nc.dram_tensor("wq", [C, HPC * Dh], BF16, kind="ExternalInput")
    wkv = nc.dram_tensor("wkv", [128, NK, 2 * Dl], BF16, kind="ExternalInput")
    eT = nc.dram_tensor("eT", [Dh, HPC * Dl], BF16, kind="ExternalInput")
    eT2 = nc.dram_tensor("eT2", [Dh, HPC * Dl], BF16, kind="ExternalInput")
    vxT = nc.dram_tensor("vxT", [Dh, HPC * Dl], BF16, kind="ExternalInput")
    pw = nc.dram_tensor("pw", [HPC * Dh, C], BF16, kind="ExternalInput")
    cosq = nc.dram_tensor("cosq", [Dh, T], F32, kind="ExternalInput")
    sinq = nc.dram_tensor("sinq", [Dh, T], F32, kind="ExternalInput")
    cosk = nc.dram_tensor("cosk", [Dl, T], F32, kind="ExternalInput")
    sink = nc.dram_tensor("sink", [Dl, T], F32, kind="ExternalInput")
    sperm = nc.dram_tensor("sperm", [Dl, Dl], F32R, kind="ExternalInput")
    ident = nc.dram_tensor("ident", [128, 128], BF16, kind="ExternalInput")
    maskt = nc.dram_tensor("maskt", [128, 4, TJ], BF16, kind="ExternalInput")
    out = nc.dram_tensor("out", [T, C], BF16, kind="ExternalOutput")

    with tile.TileContext(nc) as tc, \
         nc.allow_low_precision(reason="bf16 pipeline, fp32 accumulation"):
        cp = tc.alloc_tile_pool(name="consts", bufs=1)
        wp = tc.alloc_tile_pool(name="work", bufs=1)
        ps = tc.alloc_tile_pool(name="ps", bufs=1, space="PSUM")

        # ------- static tiles -------
        wq_sb = cp.tile([128, NK, HPC * Dh], BF16, name="wq_sb")
        wkv_sb = cp.tile([128, NK, 2 * Dl], BF16, name="wkv_sb")
        eT_sb = cp.tile([Dh, HPC * Dl], BF16, name="eT_sb")
        eT2_sb = cp.tile([Dh, HPC * Dl], BF16, name="eT2_sb")
        vxT_sb = cp.tile([Dh, HPC * Dl], BF16, name="vxT_sb")
        pw_sb = cp.tile([128, HPC, C], BF16, name="pw_sb")
        cosq_sb = cp.tile([Dh, T], F32, name="cosq_sb")
        sinq_sb = cp.tile([Dh, T], F32, name="sinq_sb")
        cosk_sb = cp.tile([Dl, T], F32, name="cosk_sb")
        sink_sb = cp.tile([Dl, T], F32, name="sink_sb")
        sperm_sb = cp.tile([Dl, Dl], F32R, name="sperm_sb")
        ident_sb = cp.tile([128, 128], BF16, name="ident_sb")
        maskt_sb = cp.tile([128, 4, TJ], BF16, name="maskt_sb")
        kk_sb = cp.tile([Dl, T], BF16, name="kk_sb")
        qtil_sb = cp.tile([Dl, HPC, T], BF16, name="qtil_sb")
        vaug_sb = cp.tile([128, NTK, Dl + 1], BF16, name="vaug_sb")
        w2_sb = cp.tile([128, 2, C], BF16, name="w2_sb")

        # ------- input DMAs (SP queue, in priority order) -------
        xr = xT[:].rearrange("(ko p) t -> p ko t", p=128)
        xts = []
        for j in range(NJ):
            xts.append(wp.tile([128, NK, TJ], BF16, name=f"xt{j}", tag="xt",
                               bufs=2))
        nc.sync.dma_start(wkv_sb, wkv[:])
        for q in range(4):
            nc.sync.dma_start(xts[0][:, 4 * q:4 * q + 4, :],
                              xr[:, 4 * q:4 * q + 4, 0:TJ])
        nc.sync.dma_start(wq_sb, wq[:].rearrange("(ko p) m -> p ko m", p=128))
        nc.sync.dma_start(cosk_sb, cosk[:])
        nc.sync.dma_start(sink_sb, sink[:])
        nc.sync.dma_start(sperm_sb, sperm[:])
        nc.sync.dma_start(cosq_sb, cosq[:])
        nc.sync.dma_start(sinq_sb, sinq[:])
        nc.sync.dma_start(eT_sb, eT[:])
        nc.sync.dma_start(eT2_sb, eT2[:])
        nc.sync.dma_start(maskt_sb, maskt[:])
        nc.sync.dma_start(ident_sb, ident[:])
        nc.sync.dma_start(vxT_sb, vxT[:])
        nc.sync.dma_start(pw_sb, pw[:].rearrange("(ko p) n -> p ko n", p=128))
        nc.vector.memset(vaug_sb[:, :, Dl:Dl + 1], 1.0)

        # ------- W2 = v_expand @ proj_w (fused output weight) -------
        for h in range(HPC):
            hs = slice((h % 2) * Dl, (h % 2) * Dl + Dl)
            for cc in range(4):
                cs = slice(cc * TJ, (cc + 1) * TJ)
                w2p = ps.tile([Dl, TJ], F32, name=f"w2p{h}_{cc}", tag="p64",
                              bufs=1)
                nc.tensor.matmul(w2p, vxT_sb[:, h * Dl:(h + 1) * Dl],
                                 pw_sb[:, h, cs], start=True, stop=True)
                nc.scalar.copy(w2_sb[hs, h // 2, cs], w2p)

        # ------- main pipeline over Tq chunks -------
        for j in range(NJ):
            js = slice(j * TJ, (j + 1) * TJ)
            if j + 1 < NJ:
                nc.sync.dma_start(xts[j + 1],
                                  xr[:, :, (j + 1) * TJ:(j + 2) * TJ])

            # --- latent K/V projection + k-RoPE ---
            kvps = ps.tile([128, TJ], F32, name=f"kvps{j}", tag="p128", bufs=2)
            for k in range(NK):
                nc.tensor.matmul(kvps, wkv_sb[:, k, :], xts[j][:, k, :],
                                 start=(k == 0), stop=(k == NK - 1))
            klat = wp.tile([Dl, TJ], F32R, name=f"klat{j}", tag="klat", bufs=2)
            nc.vector.tensor_copy(klat, kvps[0:Dl, :])
            vt = wp.tile([Dl, TJ], BF16, name=f"vt{j}", tag="vt", bufs=2)
            nc.vector.tensor_copy(vt, kvps[Dl:128, :])
            ksps = ps.tile([Dl, TJ], F32, name=f"ksps{j}", tag="p64", bufs=1)
            nc.tensor.matmul(ksps, sperm_sb, klat, start=True, stop=True)
            tk1 = wp.tile([Dl, TJ], F32, name=f"tk1_{j}", tag="tk1", bufs=2)
            nc.gpsimd.tensor_mul(tk1, klat, cosk_sb[:, js])
            tk2 = wp.tile([Dl, TJ], F32, name=f"tk2_{j}", tag="tk2", bufs=2)
            nc.vector.tensor_mul(tk2, ksps, sink_sb[:, js])
            nc.gpsimd.tensor_add(kk_sb[:, js], tk1, tk2)
            for t in range(4):
                vtt = wp.tile([128, Dl], BF16, name=f"vtt{j}_{t}", tag="vtt",
                              bufs=3)
                nc.sync.dma_start_transpose(vtt, vt[:, t * 128:(t + 1) * 128])
                nc.gpsimd.tensor_copy(vaug_sb[:, 4 * j + t, 0:Dl], vtt)

            # --- q projection + fused RoPE/absorb ---
            for h in range(HPC):
                qp = ps.tile([128, TJ], F32, name=f"qp{j}_{h}", tag="p128",
                             bufs=2)
                for k in range(NK):
                    nc.tensor.matmul(qp, wq_sb[:, k, h * 128:(h + 1) * 128],
                                     xts[j][:, k, :],
                                     start=(k == 0), stop=(k == NK - 1))
                u1 = wp.tile([128, TJ], BF16, name=f"u1_{j}_{h}", tag="u1",
                             bufs=2)
                nc.vector.tensor_mul(u1, qp, cosq_sb[:, js])
                u2 = wp.tile([128, TJ], BF16, name=f"u2_{j}_{h}", tag="u2",
                             bufs=2)
                nc.vector.tensor_mul(u2, qp, sinq_sb[:, js])
                qtp = ps.tile([Dl, TJ], F32, name=f"qtp{j}_{h}", tag="p64",
                              bufs=1)
                msl = slice(h * Dl, (h + 1) * Dl)
                nc.tensor.matmul(qtp, eT_sb[:, msl], u1,
                                 start=True, stop=False)
                nc.tensor.matmul(qtp, eT2_sb[:, msl], u2,
                                 start=False, stop=True)
                nc.vector.tensor_copy(qtil_sb[:, h, js], qtp)

            # --- attention (scores + AV in natural layout) ---
            nm = 4 * (j + 1)
            for h in range(HPC):
                p, hf = h // 2, h % 2
                av = ps.tile([128, 4, Dl + 1], F32, name=f"av{j}_{h}",
                             tag="av", bufs=1)
                if hf == 0:
                    ytp = ps.tile([128, TJ], BF16, name=f"ytp{j}_{p}",
                                  tag="ytp", bufs=1)
                exs = []
                for m in range(nm):
                    ms = slice(m * 128, (m + 1) * 128)
                    d = m - 4 * j
                    lo = 128 * d if d > 0 else 0
                    sps = ps.tile([128, TJ], F32, name=f"sps{j}_{h}_{m}",
                                  tag="sps", bufs=2)
                    nc.tensor.matmul(sps[:, lo:TJ], kk_sb[:, ms],
                                     qtil_sb[:, h, j * TJ + lo:(j + 1) * TJ],
                                     start=True, stop=True)
                    ex = wp.tile([128, TJ], BF16, name=f"ex{j}_{h}_{m}",
                                 tag="ex", bufs=24)
                    nc.scalar.activation(ex[:, lo:TJ], sps[:, lo:TJ], EXP,
                                         scale=SCALE)
                    if d >= 0:
                        band = slice(lo, lo + 128)
                        eng = nc.vector if m % 2 else nc.gpsimd
                        eng.tensor_mul(ex[:, band], ex[:, band],
                                       maskt_sb[:, d, band])
                    exs.append(ex)
                # one open accumulation chain per PSUM bank at a time:
                # issue each tq-chunk's chain contiguously (chain-major)
                for t in range(4):
                    for m in range(4 * j + t + 1):
                        nc.tensor.matmul(
                            av[:, t, :], exs[m][:, t * 128:(t + 1) * 128],
                            vaug_sb[:, m, :],
                            start=(m == 0), stop=(m == 4 * j + t))
                # normalize + transpose into ytp (pair-stacked)
                rec = wp.tile([128, 4], F32, name=f"rec{j}_{h}", tag="rec",
                              bufs=2)
                nc.vector.reciprocal(rec, av[:, :, Dl])
                yun = wp.tile([128, 4, Dl], BF16, name=f"yun{j}_{h}",
                              tag="yun", bufs=2)
                for t in range(4):
                    nc.vector.tensor_scalar_mul(yun[:, t, :], av[:, t, 0:Dl],
                                                rec[:, t:t + 1])
                    nc.tensor.transpose(
                        ytp[hf * Dl:(hf + 1) * Dl, t * 128:(t + 1) * 128],
                        yun[:, t, :], ident_sb)
                if hf == 1:
                    yt2 = wp.tile([128, TJ], BF16, name=f"yt2_{j}_{p}",
                                  tag="yt2", bufs=2)
                    nc.vector.tensor_copy(yt2, ytp)
                    if p == 0:
                        yt2a = yt2
                    else:
                        yt2b = yt2

            # --- fused output projection for this j's 4 row chunks ---
            for t in range(4):
                ts = slice(t * 128, (t + 1) * 128)
                rs = slice((4 * j + t) * 128, (4 * j + t + 1) * 128)
                ot = wp.tile([128, C], BF16, name=f"ot{j}_{t}", tag="ot",
                             bufs=2)
                for cc in range(4):
                    cs = slice(cc * TJ, (cc + 1) * TJ)
                    pps = ps.tile([128, TJ], F32, name=f"pps{j}_{t}_{cc}",
                                  tag="pps", bufs=1)
                    nc.tensor.matmul(pps, yt2a[:, ts], w2_sb[:, 0, cs],
                                     start=True, stop=False)
                    nc.tensor.matmul(pps, yt2b[:, ts], w2_sb[:, 1, cs],
                                     start=False, stop=True)
                    if (t + cc) % 2:
                        nc.scalar.copy(ot[:, cs], pps)
                    else:
                        nc.vector.tensor_copy(ot[:, cs], pps)
                nc.scalar.dma_start(out[rs, :], ot)

        ps.release()
        wp.release()
        cp.release()

    nc.compile()
    return nc


def _rope_tables(t, d):
    inv = 1.0 / (10000.0 ** (np.arange(0, d, 2, dtype=np.float64) / d))
    ang = np.arange(t, dtype=np.float64)[:, None] * inv[None, :]  # (t, d/2)
    cos = np.cos(ang).T  # (d/2, t)
    sin = np.sin(ang).T
    cosf = np.empty((d, t), np.float32)
    sinf = np.empty((d, t), np.float32)
    cosf[0::2] = cos
    cosf[1::2] = cos
    sinf[0::2] = -sin
    sinf[1::2] = sin
    return cosf, sinf


def _host_inputs(x, wq, wk_lat, wv_lat, k_expand, v_expand, proj_w):
    bf = ml_dtypes.bfloat16
    cosq, sinq = _rope_tables(T, Dh)
    sinq = np.ascontiguousarray(sinq[np.arange(Dh) ^ 1, :])  # row-pair swap
    cosk, sink = _rope_tables(T, Dl)
    sperm = np.zeros((Dl, Dl), np.float32)
    idx = np.arange(Dl)
    sperm[idx, idx ^ 1] = 1.0
    idxh = np.arange(Dh)
    ident = np.eye(128, dtype=bf)
    tkr = np.arange(128)[:, None]
    tqr = np.arange(TJ)[None, :]
    maskt = np.stack(
        [(tkr + 128 * d <= tqr).astype(bf) for d in range(4)], axis=1)
    wkv = np.concatenate([wk_lat, wv_lat], axis=1).astype(bf)  # [C, 128]
    wkvp = np.ascontiguousarray(wkv.reshape(NK, 128, 2 * Dl).transpose(1, 0, 2))

    xTs = [np.ascontiguousarray(x[b].T.astype(bf)) for b in range(B)]
    in_maps = []
    for core in range(NCORES):
        b, g = divmod(core, 4)
        heads = range(4 * g, 4 * g + 4)
        eTc = np.ascontiguousarray(
            np.concatenate([k_expand[h].T for h in heads], axis=1).astype(bf))
        eT2c = np.ascontiguousarray(eTc[idxh ^ 1, :])
        vxTc = np.ascontiguousarray(
            np.concatenate([v_expand[h].T for h in heads], axis=1).astype(bf))
        in_maps.append({
            "xT": xTs[b],
            "wq": np.ascontiguousarray(
                wq[:, g * 512:(g + 1) * 512].astype(bf)),
            "wkv": wkvp,
            "eT": eTc, "eT2": eT2c, "vxT": vxTc,
            "pw": np.ascontiguousarray(
                proj_w[g * 512:(g + 1) * 512, :].astype(bf)),
            "cosq": cosq, "sinq": sinq, "cosk": cosk, "sink": sink,
            "sperm": sperm, "ident": ident, "maskt": maskt,
        })
    return in_maps


_NC_CACHE = {}


def run(inputs, trace=False, **kw):
    """Run on all 8 cores; returns (output, BassKernelResults)."""
    if "nc" not in _NC_CACHE:
        _NC_CACHE["nc"] = build_nc()
    nc = _NC_CACHE["nc"]
    in_maps = _host_inputs(**inputs)
    res = run_bass_kernel_spmd(
        nc, in_maps, core_ids=list(range(NCORES)), trace=trace, **kw)
    out = np.zeros((B, T, C), np.float32)
    for core in range(NCORES):
        out[core // 4] += res.results[core]["out"].astype(np.float32)
    return out, res


def kernel(**inputs):
    out, _ = run(inputs)
    return out


# revision 13
# speedup vs baseline: 1.1952x; 1.0140x over previous
"""Trainium2 Bass kernel for causal MLA self-attention.

Problem: B=2, T=2048, C=2048, H=16 heads, Dh=128, latent Dl=64.
  q = rope(x @ wq); k_lat = rope(x @ wk_lat); v_lat = x @ wv_lat
  k_h = k_lat @ k_expand[h]; v_h = v_lat @ v_expand[h]
  y = causal_softmax(q k^T / sqrt(Dh)) v;  out = y @ proj_w

Sharding: 8 cores = 2 batches x 4 head-groups (4 heads each).  Each core
computes a full (T, C) partial of the output projection restricted to its
heads; the host sums the 4 partials per batch.

Device algorithm (per core), MLA absorption + W2 fusion:
  qt_h = rope(q_h) @ k_expand[h]^T             (T, 64) bf16
  s^T  = kk_rope @ qt_h^T                      (Tk, Tq) tiles; exp on ScalarE
  yu   = ex^T [stationary] @ [v_lat | 1]       (Tq128, 65) natural layout
  yun  = yu[:, :64] * recip(yu[:, 64])         per-partition scalar mul
  yt   = yun^T (PE transpose, 2 heads stacked) (128, Tq)
  out += yt2^T @ W2  where W2 = v_expand @ proj_w (fused, built on device)
All big matmul *moving* operands are bf16 (cost-model 1 cyc/row at any
width); accumulation stays fp32 in PSUM.
"""

import os
import sys

import numpy as np
import ml_dtypes

if not any(os.path.isdir(os.path.join(p, "concourse")) for p in sys.path if p):
    sys.path.insert(0, "/opt/trn_rl_repo")

import concourse.bass as bass  # noqa: E402
import concourse.mybir as mybir  # noqa: E402
import concourse.tile as tile  # noqa: E402
from concourse import bacc  # noqa: E402
from concourse.bass_utils import run_bass_kernel_spmd  # noqa: E402

B, T, C, H, Dh, Dl = 2, 2048, 2048, 16, 128, 64
HPC = 4  # heads per core
NCORES = 8
F32 = mybir.dt.float32
F32R = mybir.dt.float32r
BF16 = mybir.dt.bfloat16
SCALE = 1.0 / float(np.sqrt(Dh))

TJ = 512          # Tq chunk
NJ = T // TJ      # 4
NK = C // 128     # 16 contraction chunks over C
NTK = T // 128    # 16 Tk chunks
EXP = mybir.ActivationFunctionType.Exp


PHASE_MARKS = []


def _mark(nc, label):
    nm = nc.get_next_instruction_name()
    PHASE_MARKS.append((label, int(nm.split("-")[1])))


def build_nc():
    del PHASE_MARKS[:]
    nc = bacc.Bacc(None, target_bir_lowering=False, debug=False)

    xT = nc.dram_tensor("xT", [C, T], BF16, kind="ExternalInput")
    wq = nc.dram_tensor("wq", [C, HPC * Dh], BF16, kind="ExternalInput")
    wkv = nc.dram_tensor("wkv", [128, NK, 2 * Dl], BF16, kind="ExternalInput")
    eT = nc.dram_tensor("eT", [Dh, HPC * Dl], BF16, kind="ExternalInput")
    eT2 = nc.dram_tensor("eT2", [Dh, HPC * Dl], BF16, kind="ExternalInput")
    vxT = nc.dram_tensor("vxT", [Dh, HPC, 128], BF16, kind="ExternalInput")
    pw = nc.dram_tensor("pw", [HPC * Dh, C], BF16, kind="ExternalInput")
    cosq = nc.dram_tensor("cosq", [Dh, T], F32, kind="ExternalInput")
    sinq = nc.dram_tensor("sinq", [Dh, T], F32, kind="ExternalInput")
    cosk = nc.dram_tensor("cosk", [Dl, T], F32, kind="ExternalInput")
    sink = nc.dram_tensor("sink", [Dl, T], F32, kind="ExternalInput")
    sperm = nc.dram_tensor("sperm", [Dl, Dl], F32R, kind="ExternalInput")
    ident = nc.dram_tensor("ident", [128, 128], BF16, kind="ExternalInput")
    maskt = nc.dram_tensor("maskt", [128, 4, TJ], BF16, kind="ExternalInput")
    out = nc.dram_tensor("out", [T, C], BF16, kind="ExternalOutput")

    with tile.TileContext(nc) as tc, \
         nc.allow_low_precision(reason="bf16 pipeline, fp32 accumulation"):
        cp = tc.alloc_tile_pool(name="consts", bufs=1)
        wp = tc.alloc_tile_pool(name="work", bufs=1)
        ps = tc.alloc_tile_pool(name="ps", bufs=1, space="PSUM")

        # ------- static tiles -------
        wq_sb = cp.tile([128, NK, HPC * Dh], BF16, name="wq_sb")
        wkv_sb = cp.tile([128, NK, 2 * Dl], BF16, name="wkv_sb")
        eT_sb = cp.tile([Dh, HPC * Dl], BF16, name="eT_sb")
        eT2_sb = cp.tile([Dh, HPC * Dl], BF16, name="eT2_sb")
        vxT_sb = cp.tile([Dh, HPC, 128], BF16, name="vxT_sb")
        pw_sb = cp.tile([128, HPC, C], BF16, name="pw_sb")
        cosq_sb = cp.tile([Dh, T], F32, name="cosq_sb")
        sinq_sb = cp.tile([Dh, T], F32, name="sinq_sb")
        cosk_sb = cp.tile([Dl, T], F32, name="cosk_sb")
        sink_sb = cp.tile([Dl, T], F32, name="sink_sb")
        sperm_sb = cp.tile([Dl, Dl], F32R, name="sperm_sb")
        ident_sb = cp.tile([128, 128], BF16, name="ident_sb")
        maskt_sb = cp.tile([128, 4, TJ], BF16, name="maskt_sb")
        kk_sb = cp.tile([Dl, T], BF16, name="kk_sb")
        qtil_sb = cp.tile([Dl, HPC, T], BF16, name="qtil_sb")
        vaug_sb = cp.tile([128, NTK, Dl + 1], BF16, name="vaug_sb")
        w2_sb = cp.tile([128, 2, C], BF16, name="w2_sb")

        # ------- input DMAs (SP queue, in priority order) -------
        xr = xT[:].rearrange("(ko p) t -> p ko t", p=128)
        xts = []
        for j in range(NJ):
            xts.append(wp.tile([128, NK, TJ], BF16, name=f"xt{j}", tag="xt",
                               bufs=3))
        nc.sync.dma_start(wkv_sb, wkv[:])
        for q in range(4):
            nc.sync.dma_start(xts[0][:, 4 * q:4 * q + 4, :],
                              xr[:, 4 * q:4 * q + 4, 0:TJ])
        nc.sync.dma_start(sperm_sb, sperm[:])
        nc.sync.dma_start(cosk_sb, cosk[:])
        nc.sync.dma_start(sink_sb, sink[:])
        nc.sync.dma_start(pw_sb, pw[:].rearrange("(ko p) n -> p ko n", p=128))
        nc.sync.dma_start(vxT_sb, vxT[:])
        nc.sync.dma_start(wq_sb, wq[:].rearrange("(ko p) m -> p ko m", p=128))
        nc.sync.dma_start(cosq_sb, cosq[:])
        nc.sync.dma_start(sinq_sb, sinq[:])
        nc.sync.dma_start(eT_sb, eT[:])
        nc.sync.dma_start(eT2_sb, eT2[:])
        nc.sync.dma_start(maskt_sb, maskt[:])
        nc.sync.dma_start(ident_sb, ident[:])
        nc.vector.memset(vaug_sb[:, :, Dl:Dl + 1], 1.0)

        # ------- W2 = v_expand @ proj_w (fused output weight) -------
        for p in range(2):
            for cc in range(4):
                cs = slice(cc * TJ, (cc + 1) * TJ)
                w2p = ps.tile([128, TJ], F32, name=f"w2p{p}_{cc}", tag="sps",
                              bufs=2)
                nc.tensor.matmul(w2p, vxT_sb[:, 2 * p, :],
                                 pw_sb[:, 2 * p, cs], start=True, stop=False)
                nc.tensor.matmul(w2p, vxT_sb[:, 2 * p + 1, :],
                                 pw_sb[:, 2 * p + 1, cs],
                                 start=False, stop=True)
                nc.scalar.copy(w2_sb[:, p, cs], w2p)

        _mark(nc, "w2_done")
        # ------- main pipeline over Tq chunks -------
        for j in range(NJ):
            _mark(nc, f"j{j}_ph1")
            js = slice(j * TJ, (j + 1) * TJ)
            for jn in (j + 1, j + 2):
                if jn < NJ and (jn == j + 1 or j == 0):
                    nc.sync.dma_start(xts[jn],
                                      xr[:, :, jn * TJ:(jn + 1) * TJ])

            # --- latent K/V projection + k-RoPE ---
            kvps = ps.tile([128, TJ], F32, name=f"kvps{j}", tag="p128", bufs=2)
            for k in range(NK):
                nc.tensor.matmul(kvps, wkv_sb[:, k, :], xts[j][:, k, :],
                                 start=(k == 0), stop=(k == NK - 1))
            klat = wp.tile([Dl, TJ], F32R, name=f"klat{j}", tag="klat", bufs=2)
            nc.vector.tensor_copy(klat, kvps[0:Dl, :])
            vt = wp.tile([Dl, TJ], BF16, name=f"vt{j}", tag="vt", bufs=2)
            nc.vector.tensor_copy(vt, kvps[Dl:128, :])
            ksps = ps.tile([Dl, TJ], F32, name=f"ksps{j}", tag="p64", bufs=1)
            nc.tensor.matmul(ksps, sperm_sb, klat, start=True, stop=True)
            tk1 = wp.tile([Dl, TJ], F32, name=f"tk1_{j}", tag="tk1", bufs=2)
            nc.gpsimd.tensor_mul(tk1, klat, cosk_sb[:, js])
            tk2 = wp.tile([Dl, TJ], F32, name=f"tk2_{j}", tag="tk2", bufs=2)
            nc.vector.tensor_mul(tk2, ksps, sink_sb[:, js])
            nc.gpsimd.tensor_add(kk_sb[:, js], tk1, tk2)
            for t in range(4):
                vtt = wp.tile([128, Dl], BF16, name=f"vtt{j}_{t}", tag="vtt",
                              bufs=3)
                nc.sync.dma_start_transpose(vtt, vt[:, t * 128:(t + 1) * 128])
                nc.gpsimd.tensor_copy(vaug_sb[:, 4 * j + t, 0:Dl], vtt)

            # --- q projection + fused RoPE/absorb ---
            for h in range(HPC):
                qp = ps.tile([128, TJ], F32, name=f"qp{j}_{h}", tag="p128",
                             bufs=2)
                for k in range(NK):
                    nc.tensor.matmul(qp, wq_sb[:, k, h * 128:(h + 1) * 128],
                                     xts[j][:, k, :],
                                     start=(k == 0), stop=(k == NK - 1))
                qsb = wp.tile([128, TJ], BF16, name=f"qsb{j}_{h}", tag="qsb",
                              bufs=2)
                nc.vector.tensor_copy(qsb, qp)
                u1 = wp.tile([128, TJ], BF16, name=f"u1_{j}_{h}", tag="u1",
                             bufs=2)
                nc.gpsimd.tensor_mul(u1, qsb, cosq_sb[:, js])
                u2 = wp.tile([128, TJ], BF16, name=f"u2_{j}_{h}", tag="u2",
                             bufs=2)
                nc.gpsimd.tensor_mul(u2, qsb, sinq_sb[:, js])
                qtp = ps.tile([Dl, TJ], F32, name=f"qtp{j}_{h}", tag="p64",
                              bufs=1)
                msl = slice(h * Dl, (h + 1) * Dl)
                nc.tensor.matmul(qtp, eT_sb[:, msl], u1,
                                 start=True, stop=False)
                nc.tensor.matmul(qtp, eT2_sb[:, msl], u2,
                                 start=False, stop=True)
                nc.vector.tensor_copy(qtil_sb[:, h, js], qtp)

            # --- attention (scores + AV in natural layout) ---
            _mark(nc, f"j{j}_att")
            nm = 4 * (j + 1)
            for h in range(HPC):
                p, hf = h // 2, h % 2
                av = ps.tile([128, 4, Dl + 1], F32, name=f"av{j}_{h}",
                             tag="av", bufs=1)
                if hf == 0:
                    ytp = ps.tile([128, TJ], BF16, name=f"ytp{j}_{p}",
                                  tag="ytp", bufs=1)
                exs = []
                for m in range(nm):
                    ms = slice(m * 128, (m + 1) * 128)
                    d = m - 4 * j
                    lo = 128 * d if d > 0 else 0
                    sps = ps.tile([128, TJ], F32, name=f"sps{j}_{h}_{m}",
                                  tag="sps", bufs=2)
                    nc.tensor.matmul(sps[:, lo:TJ], kk_sb[:, ms],
                                     qtil_sb[:, h, j * TJ + lo:(j + 1) * TJ],
                                     start=True, stop=True)
                    ex = wp.tile([128, TJ], BF16, name=f"ex{j}_{h}_{m}",
                                 tag="ex", bufs=22)
                    nc.scalar.activation(ex[:, lo:TJ], sps[:, lo:TJ], EXP,
                                         scale=SCALE)
                    if d >= 0:
                        band = slice(lo, lo + 128)
                        nc.gpsimd.tensor_mul(ex[:, band], ex[:, band],
                                              maskt_sb[:, d, band])
                    exs.append(ex)
                # one open accumulation chain per PSUM bank at a time:
                # issue each tq-chunk's chain contiguously (chain-major)
                for t in range(4):
                    for m in range(4 * j + t + 1):
                        nc.tensor.matmul(
                            av[:, t, :], exs[m][:, t * 128:(t + 1) * 128],
                            vaug_sb[:, m, :],
                            start=(m == 0), stop=(m == 4 * j + t))
                # normalize + transpose into ytp (pair-stacked)
                rec = wp.tile([128, 4], F32, name=f"rec{j}_{h}", tag="rec",
                              bufs=2)
                nc.vector.reciprocal(rec, av[:, :, Dl])
                yun = wp.tile([128, 4, Dl], BF16, name=f"yun{j}_{h}",
                              tag="yun", bufs=2)
                for t in range(4):
                    nc.vector.tensor_scalar_mul(yun[:, t, :], av[:, t, 0:Dl],
                                                rec[:, t:t + 1])
                    nc.tensor.transpose(
                        ytp[hf * Dl:(hf + 1) * Dl, t * 128:(t + 1) * 128],
                        yun[:, t, :], ident_sb)
                if hf == 1:
                    yt2 = wp.tile([128, TJ], BF16, name=f"yt2_{j}_{p}",
                                  tag="yt2", bufs=2)
                    nc.vector.tensor_copy(yt2, ytp)
                    if p == 0:
                        yt2a = yt2
                    else:
                        yt2b = yt2

            _mark(nc, f"j{j}_proj")
            # --- fused output projection for this j's 4 row chunks ---
            for t in range(4):
                ts = slice(t * 128, (t + 1) * 128)
                rs = slice((4 * j + t) * 128, (4 * j + t + 1) * 128)
                ot = wp.tile([128, C], BF16, name=f"ot{j}_{t}", tag="ot",
                             bufs=2)
                for cc in range(4):
                    cs = slice(cc * TJ, (cc + 1) * TJ)
                    pps = ps.tile([128, TJ], F32, name=f"pps{j}_{t}_{cc}",
                                  tag="pps", bufs=1)
                    nc.tensor.matmul(pps, yt2a[:, ts], w2_sb[:, 0, cs],
                                     start=True, stop=False)
                    nc.tensor.matmul(pps, yt2b[:, ts], w2_sb[:, 1, cs],
                                     start=False, stop=True)
                    nc.vector.tensor_copy(ot[:, cs], pps)
                nc.scalar.dma_start(out[rs, :], ot)

        _mark(nc, "end")
        ps.release()
        wp.release()
        cp.release()

    nc.compile()
    return nc


def _rope_tables(t, d):
    inv = 1.0 / (10000.0 ** (np.arange(0, d, 2, dtype=np.float64) / d))
    ang = np.arange(t, dtype=np.float64)[:, None] * inv[None, :]  # (t, d/2)
    cos = np.cos(ang).T  # (d/2, t)
    sin = np.sin(ang).T
    cosf = np.empty((d, t), np.float32)
    sinf = np.empty((d, t), np.float32)
    cosf[0::2] = cos
    cosf[1::2] = cos
    sinf[0::2] = -sin
    sinf[1::2] = sin
    return cosf, sinf


def _host_inputs(x, wq, wk_lat, wv_lat, k_expand, v_expand, proj_w):
    bf = ml_dtypes.bfloat16
    cosq, sinq = _rope_tables(T, Dh)
    sinq = np.ascontiguousarray(sinq[np.arange(Dh) ^ 1, :])  # row-pair swap
    cosk, sink = _rope_tables(T, Dl)
    sperm = np.zeros((Dl, Dl), np.float32)
    idx = np.arange(Dl)
    sperm[idx, idx ^ 1] = 1.0
    idxh = np.arange(Dh)
    ident = np.eye(128, dtype=bf)
    tkr = np.arange(128)[:, None]
    tqr = np.arange(TJ)[None, :]
    maskt = np.stack(
        [(tkr + 128 * d <= tqr).astype(bf) for d in range(4)], axis=1)
    wkv = np.concatenate([wk_lat, wv_lat], axis=1).astype(bf)  # [C, 128]
    wkvp = np.ascontiguousarray(wkv.reshape(NK, 128, 2 * Dl).transpose(1, 0, 2))

    xTs = [np.ascontiguousarray(x[b].T.astype(bf)) for b in range(B)]
    in_maps = []
    for core in range(NCORES):
        b, g = divmod(core, 4)
        heads = range(4 * g, 4 * g + 4)
        eTc = np.ascontiguousarray(
            np.concatenate([k_expand[h].T for h in heads], axis=1).astype(bf))
        eT2c = np.ascontiguousarray(eTc[idxh ^ 1, :])
        vxTc = np.zeros((Dh, HPC, 128), bf)
        for i, h in enumerate(heads):
            vxTc[:, i, (i % 2) * Dl:(i % 2) * Dl + Dl] = v_expand[h].T
        in_maps.append({
            "xT": xTs[b],
            "wq": np.ascontiguousarray(
                wq[:, g * 512:(g + 1) * 512].astype(bf)),
            "wkv": wkvp,
            "eT": eTc, "eT2": eT2c, "vxT": vxTc,
            "pw": np.ascontiguousarray(
                proj_w[g * 512:(g + 1) * 512, :].astype(bf)),
            "cosq": cosq, "sinq": sinq, "cosk": cosk, "sink": sink,
            "sperm": sperm, "ident": ident, "maskt": maskt,
        })
    return in_maps


_NC_CACHE = {}


def run(inputs, trace=False, **kw):
    """Run on all 8 cores; returns (output, BassKernelResults)."""
    if "nc" not in _NC_CACHE:
        _NC_CACHE["nc"] = build_nc()
    nc = _NC_CACHE["nc"]
    in_maps = _host_inputs(**inputs)
    res = run_bass_kernel_spmd(
        nc, in_maps, core_ids=list(range(NCORES)), trace=trace, **kw)
    out = np.zeros((B, T, C), np.float32)
    for core in range(NCORES):
        out[core // 4] += res.results[core]["out"].astype(np.float32)
    return out, res


def kernel(**inputs):
    out, _ = run(inputs)
    return out


# revision 22
# speedup vs baseline: 1.2738x; 1.0657x over previous
"""Trainium2 Bass kernel for causal MLA self-attention.

Problem: B=2, T=2048, C=2048, H=16 heads, Dh=128, latent Dl=64.
  q = rope(x @ wq); k_lat = rope(x @ wk_lat); v_lat = x @ wv_lat
  k_h = k_lat @ k_expand[h]; v_h = v_lat @ v_expand[h]
  y = causal_softmax(q k^T / sqrt(Dh)) v;  out = y @ proj_w

Sharding: 8 cores = 2 batches x 4 head-groups (4 heads each).  Each core
computes a full (T, C) partial of the output projection restricted to its
heads; the host sums the 4 partials per batch.

Device algorithm (per core), MLA absorption + W2 fusion:
  qt_h = rope(q_h) @ k_expand[h]^T             (T, 64) bf16
  s^T  = kk_rope @ qt_h^T                      (Tk, Tq) tiles; exp on ScalarE
  yu   = ex^T [stationary] @ [v_lat | 1]       (Tq128, 65) natural layout
  yun  = yu[:, :64] * recip(yu[:, 64])         per-partition scalar mul
  yt   = yun^T via XBAR dma transpose          (128, Tq), 2 heads stacked
  out += yt2^T @ W2  where W2 = v_expand @ proj_w (fused, built on device)
All big matmul *moving* operands are bf16 (cost-model 1 cyc/row at any
width); accumulation stays fp32 in PSUM.  Small constants travel in one
packed bf16 blob so the xt[1] prefetch isn't stuck behind a dozen DMAs.
"""

import os
import sys

import numpy as np
import ml_dtypes

if not any(os.path.isdir(os.path.join(p, "concourse")) for p in sys.path if p):
    sys.path.insert(0, "/opt/trn_rl_repo")

import concourse.bass as bass  # noqa: E402
import concourse.mybir as mybir  # noqa: E402
import concourse.tile as tile  # noqa: E402
from concourse import bacc  # noqa: E402
from concourse.bass_utils import run_bass_kernel_spmd  # noqa: E402

B, T, C, H, Dh, Dl = 2, 2048, 2048, 16, 128, 64
HPC = 4  # heads per core
NCORES = 8
F32 = mybir.dt.float32
F32R = mybir.dt.float32r
BF16 = mybir.dt.bfloat16
SCALE = 1.0 / float(np.sqrt(Dh))

TJ = 512          # Tq chunk
NJ = T // TJ      # 4
NK = C // 128     # 16 contraction chunks over C
NTK = T // 128    # 16 Tk chunks
EXP = mybir.ActivationFunctionType.Exp

# packed-constant blob column offsets (bf16, 128 partitions)
O_COSQ = 0
O_SINQ = O_COSQ + T
O_COSK = O_SINQ + T          # cosk on partitions 0:64, sink on 64:128
O_MASK = O_COSK + T
O_ET = O_MASK + 4 * TJ
O_ET2 = O_ET + HPC * Dl
O_VXT = O_ET2 + HPC * Dl     # padded pair layout [128, 4, 128]
O_SPERM = O_VXT + HPC * 128
NCST = O_SPERM + Dl

PHASE_MARKS = []

CFG = {
    "sps_bufs": 2,
    "pps_tag": "pps",      # "pps" (own ring) | "sps" (share)
    "pps_bufs": 2,
    "p128_bufs": 2,
    "ex_bufs": 22,
    "mask_eng": "dve",     # "pool" | "dve" | "alt"
    "w2_eng": "dve",
    "yt_mode": "dma",      # "dma" (XBAR transpose) | "pe"
}


def _mark(nc, label):
    nm = nc.get_next_instruction_name()
    PHASE_MARKS.append((label, int(nm.split("-")[1])))


def build_nc():
    del PHASE_MARKS[:]
    nc = bacc.Bacc(None, target_bir_lowering=False, debug=False)

    xT = nc.dram_tensor("xT", [C, T], BF16, kind="ExternalInput")
    wq = nc.dram_tensor("wq", [C, HPC * Dh], BF16, kind="ExternalInput")
    wkv = nc.dram_tensor("wkv", [128, NK, 2 * Dl], BF16, kind="ExternalInput")
    cst = nc.dram_tensor("cst", [128, NCST], BF16, kind="ExternalInput")
    pw = nc.dram_tensor("pw", [HPC * Dh, C], BF16, kind="ExternalInput")
    out = nc.dram_tensor("out", [T, C], BF16, kind="ExternalOutput")

    with tile.TileContext(nc) as tc, \
         nc.allow_low_precision(reason="bf16 pipeline, fp32 accumulation"):
        cp = tc.alloc_tile_pool(name="consts", bufs=1)
        wp = tc.alloc_tile_pool(name="work", bufs=1)
        ps = tc.alloc_tile_pool(name="ps", bufs=1, space="PSUM")

        # ------- static tiles -------
        wq_sb = cp.tile([128, NK, HPC * Dh], BF16, name="wq_sb")
        wkv_sb = cp.tile([128, NK, 2 * Dl], BF16, name="wkv_sb")
        cst_sb = cp.tile([128, NCST], BF16, name="cst_sb")
        pw_sb = cp.tile([128, HPC, C], BF16, name="pw_sb")
        kk_sb = cp.tile([Dl, T], BF16, name="kk_sb")
        qtil_sb = cp.tile([Dl, HPC, T], BF16, name="qtil_sb")
        vaug_sb = cp.tile([128, NTK, Dl + 1], BF16, name="vaug_sb")
        w2_sb = cp.tile([128, 2, C], BF16, name="w2_sb")

        cosq_v = cst_sb[:, O_COSQ:O_COSQ + T]
        sinq_v = cst_sb[:, O_SINQ:O_SINQ + T]
        cosk_v = cst_sb[0:Dl, O_COSK:O_COSK + T]
        sink_v = cst_sb[Dl:128, O_COSK:O_COSK + T]
        sperm_v = cst_sb[0:Dl, O_SPERM:O_SPERM + Dl]

        def mask_v(d, band):
            return cst_sb[:, O_MASK + d * TJ + band.start:
                          O_MASK + d * TJ + band.stop]

        def eT_v(h):
            return cst_sb[:, O_ET + h * Dl:O_ET + (h + 1) * Dl]

        def eT2_v(h):
            return cst_sb[:, O_ET2 + h * Dl:O_ET2 + (h + 1) * Dl]

        def vxT_v(k):
            return cst_sb[:, O_VXT + k * 128:O_VXT + (k + 1) * 128]

        # ------- input DMAs (SP queue, priority order) -------
        xr = xT[:].rearrange("(ko p) t -> p ko t", p=128)
        xts = [wp.tile([128, NK, TJ], BF16, name=f"xt{j}", tag="xt", bufs=3)
               for j in range(NJ)]
        nc.sync.dma_start(wkv_sb, wkv[:])
        for q in range(2):
            nc.sync.dma_start(xts[0][:, 8 * q:8 * q + 8, :],
                              xr[:, 8 * q:8 * q + 8, 0:TJ])
        nc.sync.dma_start(wq_sb, wq[:].rearrange("(ko p) m -> p ko m", p=128))
        nc.sync.dma_start(xts[1], xr[:, :, TJ:2 * TJ])
        nc.sync.dma_start(cst_sb, cst[:])
        nc.vector.memset(vaug_sb[:, :, Dl:Dl + 1], 1.0)

        def late_dmas():
            nc.sync.dma_start(pw_sb, pw[:].rearrange("(ko p) n -> p ko n",
                                                     p=128))
            nc.sync.dma_start(xts[2], xr[:, :, 2 * TJ:3 * TJ])

        def build_w2():
            for p in range(2):
                for cc in range(4):
                    cs = slice(cc * TJ, (cc + 1) * TJ)
                    w2p = ps.tile([128, TJ], F32, name=f"w2p{p}_{cc}",
                                  tag="sps", bufs=CFG["sps_bufs"])
                    nc.tensor.matmul(w2p, vxT_v(2 * p), pw_sb[:, 2 * p, cs],
                                     start=True, stop=False)
                    nc.tensor.matmul(w2p, vxT_v(2 * p + 1),
                                     pw_sb[:, 2 * p + 1, cs],
                                     start=False, stop=True)
                    if CFG["w2_eng"] == "dve":
                        nc.vector.tensor_copy(w2_sb[:, p, cs], w2p)
                    else:
                        nc.scalar.copy(w2_sb[:, p, cs], w2p)

        yt2s = {}

        def ph1_kv(j):
            _mark(nc, f"j{j}_ph1")
            js = slice(j * TJ, (j + 1) * TJ)
            if j + 1 == 3:
                nc.sync.dma_start(xts[3], xr[:, :, 3 * TJ:4 * TJ])
            kvps = ps.tile([128, TJ], F32, name=f"kvps{j}", tag="p128",
                           bufs=CFG["p128_bufs"])
            for k in range(NK):
                nc.tensor.matmul(kvps, wkv_sb[:, k, :], xts[j][:, k, :],
                                 start=(k == 0), stop=(k == NK - 1))
            klat = wp.tile([Dl, TJ], BF16, name=f"klat{j}", tag="klat", bufs=2)
            nc.vector.tensor_copy(klat, kvps[0:Dl, :])
            vt = wp.tile([Dl, TJ], BF16, name=f"vt{j}", tag="vt", bufs=2)
            nc.vector.tensor_copy(vt, kvps[Dl:128, :])
            ksps = ps.tile([Dl, TJ], F32, name=f"ksps{j}", tag="p64", bufs=1)
            nc.tensor.matmul(ksps, sperm_v, klat, start=True, stop=True)
            tk1 = wp.tile([Dl, TJ], F32, name=f"tk1_{j}", tag="tk1", bufs=2)
            nc.gpsimd.tensor_mul(tk1, klat, cosk_v[:, js])
            tk2 = wp.tile([Dl, TJ], F32, name=f"tk2_{j}", tag="tk2", bufs=2)
            nc.vector.tensor_mul(tk2, ksps, sink_v[:, js])
            nc.gpsimd.tensor_add(kk_sb[:, js], tk1, tk2)
            for t in range(4):
                vtt = wp.tile([128, Dl], BF16, name=f"vtt{j}_{t}", tag="vtt",
                              bufs=3)
                nc.sync.dma_start_transpose(vtt, vt[:, t * 128:(t + 1) * 128])
                nc.gpsimd.tensor_copy(vaug_sb[:, 4 * j + t, 0:Dl], vtt)

        def ph1_q(j, h):
            js = slice(j * TJ, (j + 1) * TJ)
            qp = ps.tile([128, TJ], F32, name=f"qp{j}_{h}", tag="p128",
                         bufs=CFG["p128_bufs"])
            for k in range(NK):
                nc.tensor.matmul(qp, wq_sb[:, k, h * 128:(h + 1) * 128],
                                 xts[j][:, k, :],
                                 start=(k == 0), stop=(k == NK - 1))
            qsb = wp.tile([128, TJ], BF16, name=f"qsb{j}_{h}", tag="qsb",
                          bufs=2)
            nc.vector.tensor_copy(qsb, qp)
            u1 = wp.tile([128, TJ], BF16, name=f"u1_{j}_{h}", tag="u1", bufs=2)
            nc.gpsimd.tensor_mul(u1, qsb, cosq_v[:, js])
            u2 = wp.tile([128, TJ], BF16, name=f"u2_{j}_{h}", tag="u2", bufs=2)
            nc.gpsimd.tensor_mul(u2, qsb, sinq_v[:, js])
            qtp = ps.tile([Dl, TJ], F32, name=f"qtp{j}_{h}", tag="p64", bufs=1)
            nc.tensor.matmul(qtp, eT_v(h), u1, start=True, stop=False)
            nc.tensor.matmul(qtp, eT2_v(h), u2, start=False, stop=True)
            nc.vector.tensor_copy(qtil_sb[:, h, js], qtp)

        def att_head(j, h):
            if h == 0:
                _mark(nc, f"j{j}_att")
            nm = 4 * (j + 1)
            p, hf = h // 2, h % 2
            av = ps.tile([128, 4, Dl + 1], F32, name=f"av{j}_{h}",
                         tag="av", bufs=1)
            if hf == 0:
                yt2 = wp.tile([128, TJ], BF16, name=f"yt2_{j}_{p}",
                              tag="yt2", bufs=2)
                yt2s[(j, p)] = yt2
            else:
                yt2 = yt2s[(j, p)]
            exs = []
            for m in range(nm):
                ms = slice(m * 128, (m + 1) * 128)
                d = m - 4 * j
                lo = 128 * d if d > 0 else 0
                sps = ps.tile([128, TJ], F32, name=f"sps{j}_{h}_{m}",
                              tag="sps", bufs=CFG["sps_bufs"])
                nc.tensor.matmul(sps[:, lo:TJ], kk_sb[:, ms],
                                 qtil_sb[:, h, j * TJ + lo:(j + 1) * TJ],
                                 start=True, stop=True)
                ex = wp.tile([128, TJ], BF16, name=f"ex{j}_{h}_{m}",
                             tag="ex", bufs=CFG["ex_bufs"])
                nc.scalar.activation(ex[:, lo:TJ], sps[:, lo:TJ], EXP,
                                     scale=SCALE)
                if d >= 0:
                    band = slice(lo, lo + 128)
                    me = {"pool": nc.gpsimd, "dve": nc.vector,
                          "alt": (nc.vector if m % 2 else nc.gpsimd)}[
                              CFG["mask_eng"]]
                    me.tensor_mul(ex[:, band], ex[:, band], mask_v(d, band))
                exs.append(ex)
            # one open accumulation chain per PSUM bank at a time:
            # issue each tq-chunk's chain contiguously (chain-major)
            for t in range(4):
                for m in range(4 * j + t + 1):
                    nc.tensor.matmul(
                        av[:, t, :], exs[m][:, t * 128:(t + 1) * 128],
                        vaug_sb[:, m, :],
                        start=(m == 0), stop=(m == 4 * j + t))
            # normalize + transpose into yt2 (pair-stacked)
            rec = wp.tile([128, 4], F32, name=f"rec{j}_{h}", tag="rec",
                          bufs=2)
            nc.vector.reciprocal(rec, av[:, :, Dl])
            yun = wp.tile([128, 4, Dl], BF16, name=f"yun{j}_{h}",
                          tag="yun", bufs=2)
            for t in range(4):
                nc.vector.tensor_scalar_mul(yun[:, t, :], av[:, t, 0:Dl],
                                            rec[:, t:t + 1])
            for tp in (0, 2):
                yts = wp.tile([128, 128], BF16, name=f"yts{j}_{h}_{tp}",
                              tag="yts", bufs=4)
                nc.sync.dma_start_transpose(yts, yun[:, tp:tp + 2, :])
                nc.gpsimd.tensor_copy(
                    yt2[hf * Dl:(hf + 1) * Dl, tp * 128:(tp + 1) * 128],
                    yts[0:Dl, :])
                nc.gpsimd.tensor_copy(
                    yt2[hf * Dl:(hf + 1) * Dl,
                        (tp + 1) * 128:(tp + 2) * 128], yts[Dl:128, :])

        def proj_chunk(j, t):
            if t == 0:
                _mark(nc, f"j{j}_proj")
            yt2a, yt2b = yt2s[(j, 0)], yt2s[(j, 1)]
            ts = slice(t * 128, (t + 1) * 128)
            rs = slice((4 * j + t) * 128, (4 * j + t + 1) * 128)
            ot = wp.tile([128, C], BF16, name=f"ot{j}_{t}", tag="ot", bufs=2)
            for cc in range(4):
                cs = slice(cc * TJ, (cc + 1) * TJ)
                pps = ps.tile([128, TJ], F32, name=f"pps{j}_{t}_{cc}",
                              tag=CFG["pps_tag"],
                              bufs=(CFG["pps_bufs"]
                                    if CFG["pps_tag"] == "pps"
                                    else CFG["sps_bufs"]))
                nc.tensor.matmul(pps, yt2a[:, ts], w2_sb[:, 0, cs],
                                 start=True, stop=False)
                nc.tensor.matmul(pps, yt2b[:, ts], w2_sb[:, 1, cs],
                                 start=False, stop=True)
                nc.vector.tensor_copy(ot[:, cs], pps)
            nc.sync.dma_start(out[rs, :], ot)

        # software-pipelined issue: interleave attention heads with the next
        # chunk's projections and the previous chunk's output projection
        ph1_kv(0)
        late_dmas()
        for h in range(HPC):
            ph1_q(0, h)
        build_w2()
        for j in range(NJ):
            for h in range(HPC):
                att_head(j, h)
                if j + 1 < NJ:
                    if h == 0:
                        ph1_kv(j + 1)
                    else:
                        ph1_q(j + 1, h - 1)
                if j >= 1:
                    proj_chunk(j - 1, h)
            if j + 1 < NJ:
                ph1_q(j + 1, 3)
        for t in range(4):
            proj_chunk(NJ - 1, t)

        _mark(nc, "end")
        ps.release()
        wp.release()
        cp.release()

    nc.compile()
    return nc


def _rope_tables(t, d):
    inv = 1.0 / (10000.0 ** (np.arange(0, d, 2, dtype=np.float64) / d))
    ang = np.arange(t, dtype=np.float64)[:, None] * inv[None, :]  # (t, d/2)
    cos = np.cos(ang).T  # (d/2, t)
    sin = np.sin(ang).T
    cosf = np.empty((d, t), np.float32)
    sinf = np.empty((d, t), np.float32)
    cosf[0::2] = cos
    cosf[1::2] = cos
    sinf[0::2] = -sin
    sinf[1::2] = sin
    return cosf, sinf


def _host_inputs(x, wq, wk_lat, wv_lat, k_expand, v_expand, proj_w):
    bf = ml_dtypes.bfloat16
    cosq, sinq = _rope_tables(T, Dh)
    sinq = np.ascontiguousarray(sinq[np.arange(Dh) ^ 1, :])  # row-pair swap
    cosk, sink = _rope_tables(T, Dl)
    idxh = np.arange(Dh)
    tkr = np.arange(128)[:, None]
    tqr = np.arange(TJ)[None, :]
    wkv = np.concatenate([wk_lat, wv_lat], axis=1).astype(bf)  # [C, 128]
    wkvp = np.ascontiguousarray(wkv.reshape(NK, 128, 2 * Dl).transpose(1, 0, 2))

    xTs = [np.ascontiguousarray(x[b].T.astype(bf)) for b in range(B)]
    in_maps = []
    for core in range(NCORES):
        b, g = divmod(core, 4)
        heads = range(4 * g, 4 * g + 4)
        blob = np.zeros((128, NCST), bf)
        blob[:, O_COSQ:O_COSQ + T] = cosq.astype(bf)
        blob[:, O_SINQ:O_SINQ + T] = sinq.astype(bf)
        blob[0:Dl, O_COSK:O_COSK + T] = cosk.astype(bf)
        blob[Dl:128, O_COSK:O_COSK + T] = sink.astype(bf)
        for d in range(4):
            blob[:, O_MASK + d * TJ:O_MASK + (d + 1) * TJ] = \
                (tkr + 128 * d <= tqr).astype(bf)
        eTc = np.concatenate([k_expand[hh].T for hh in heads],
                             axis=1).astype(bf)
        blob[:, O_ET:O_ET + HPC * Dl] = eTc
        blob[:, O_ET2:O_ET2 + HPC * Dl] = eTc[idxh ^ 1, :]
        for i, hh in enumerate(heads):
            blob[:, O_VXT + i * 128 + (i % 2) * Dl:
                 O_VXT + i * 128 + (i % 2) * Dl + Dl] = v_expand[hh].T
        idx = np.arange(Dl)
        blob[idx, O_SPERM + (idx ^ 1)] = 1.0
        in_maps.append({
            "xT": xTs[b],
            "wq": np.ascontiguousarray(
                wq[:, g * 512:(g + 1) * 512].astype(bf)),
            "wkv": wkvp,
            "cst": blob,
            "pw": np.ascontiguousarray(
                proj_w[g * 512:(g + 1) * 512, :].astype(bf)),
        })
    return in_maps


_NC_CACHE = {}


def run(inputs, trace=False, **kw):
    """Run on all 8 cores; returns (output, BassKernelResults)."""
    if "nc" not in _NC_CACHE:
        _NC_CACHE["nc"] = build_nc()
    nc = _NC_CACHE["nc"]
    in_maps = _host_inputs(**inputs)
    res = run_bass_kernel_spmd(
        nc, in_maps, core_ids=list(range(NCORES)), trace=trace, **kw)
    out = np.zeros((B, T, C), np.float32)
    for core in range(NCORES):
        out[core // 4] += res.results[core]["out"].astype(np.float32)
    return out, res


def kernel(**inputs):
    out, _ = run(inputs)
    return out
